# revision 9
# baseline (speedup 1.0000x reference)
"""Trainium2 Bass kernel for EnhancedFRAUnifiedEncoder (kNN-graph message passing).

Sharding: batch dim B=8 across 8 cores; each core runs the 3-layer GNN locally.
Host-side trick: nodes are sorted by x-coordinate per batch (the GNN is
permutation-equivariant), which makes the kNN adjacency block-tridiagonal
(bandwidth verified on host at runtime), so the [N,N] message-passing matmul,
the pairwise-distance build, and the symmetrization all shrink to the band.
Phase A (adjacency build), A2 (symmetrize), and layer-0 message passing are
interleaved per block-row so all engines stay busy from the start.
"""
import numpy as np
from contextlib import ExitStack

import concourse.tile as tile
from concourse import bacc, mybir
from concourse import bass_utils
from concourse.masks import make_identity

F32 = mybir.dt.float32
F16 = mybir.dt.float16
AF = mybir.ActivationFunctionType
ALU = mybir.AluOpType

B = 8
N = 2048
D = 512
P = 128
NB = N // P          # 16 row blocks
NDC = D // P         # 4 feature chunks of 128
NLAYER = 3
LN_EPS = 1e-5

_CACHE = {}


def _build_nc(bw=1, triv=True):
    """bw: adjacency half-bandwidth in 128-blocks. triv: b==0, gamma==1, beta==0."""
    key = ("nc", bw, triv)
    if key in _CACHE:
        return _CACHE[key]
    nc = bacc.Bacc("TRN2", target_bir_lowering=False, debug=False, num_devices=B)

    def jlo(m):
        return max(0, m - bw)

    def jhi(m):
        return min(NB - 1, m + bw)

    def wid(m):
        return (jhi(m) - jlo(m) + 1) * P

    WMAX = (2 * bw + 1) * P

    x_d = nc.dram_tensor("xin", [N, D], F32, kind="ExternalInput").ap()
    cxb_d = nc.dram_tensor("cxb", [2, P, N], F32, kind="ExternalInput").ap()
    ccol_d = nc.dram_tensor("ccol", [2, P, NB], F32, kind="ExternalInput").ap()
    w_d = nc.dram_tensor("w16", [NLAYER, NDC, P, D], F16, kind="ExternalInput").ap()
    bgb_d = nc.dram_tensor("bgb", [NLAYER, 3, P, D], F32, kind="ExternalInput").ap()
    out_d = nc.dram_tensor("out", [N, D], F32, kind="ExternalOutput").ap()

    with tile.TileContext(nc) as tc, ExitStack() as ctx:
        # ---- persistent pools -------------------------------------------------
        apool = ctx.enter_context(tc.tile_pool(name="apool", bufs=1))
        xpool = ctx.enter_context(tc.tile_pool(name="xpool", bufs=1))
        cpool = ctx.enter_context(tc.tile_pool(name="cpool", bufs=1))

        a_t = [apool.tile([P, wid(m)], F16, name=f"a{m}", tag=f"a{m}")
               for m in range(NB)]
        x32 = [xpool.tile([P, D], F32, name=f"x32_{i}", tag=f"x32_{i}")
               for i in range(NB)]
        # double-buffered fp16 x (layer l reads buf l%2) and yT
        x16 = [[xpool.tile([P, D], F16, name=f"x16_{s}_{i}", tag=f"x16_{s}_{i}")
                for i in range(NB)] for s in range(2)]
        yt = [xpool.tile([P, NDC, N], F16, name=f"yt{s}", tag=f"yt{s}")
              for s in range(2)]
        w_sb = cpool.tile([P, NLAYER * NDC * D], F16, name="w_sb", tag="w_sb")
        bgb_sb = (None if triv else
                  cpool.tile([P, NLAYER * 3 * D], F32, name="bgb_sb", tag="bgb_sb"))
        ident16 = cpool.tile([P, P], F16, name="ident16", tag="ident16")
        eps_sb = cpool.tile([P, 1], F32, name="eps_sb", tag="eps_sb")
        nc.gpsimd.memset(eps_sb[:], LN_EPS)
        make_identity(nc, ident16[:])

        # ---- input DMAs up front (overlap with phase A) ----------------------
        bpool = ctx.enter_context(tc.tile_pool(name="bpool", bufs=1))
        cxb_sb = bpool.tile([P, 2 * N], F32, name="cxb_sb", tag="cxb_sb")
        ccol_sb = bpool.tile([P, 2 * NB], F32, name="ccol_sb", tag="ccol_sb")
        nc.sync.dma_start(out=cxb_sb[:, 0:N], in_=cxb_d[0])
        nc.scalar.dma_start(out=cxb_sb[:, N:], in_=cxb_d[1])
        nc.sync.dma_start(out=ccol_sb[:, 0:NB], in_=ccol_d[0])
        nc.sync.dma_start(out=ccol_sb[:, NB:], in_=ccol_d[1])
        for l in range(NLAYER):
            for dt in range(NDC):
                nc.sync.dma_start(
                    out=w_sb[:, (l * NDC + dt) * D:(l * NDC + dt + 1) * D],
                    in_=w_d[l, dt],
                )
            if not triv:
                for pr in range(3):
                    nc.sync.dma_start(
                        out=bgb_sb[:, (l * 3 + pr) * D:(l * 3 + pr + 1) * D],
                        in_=bgb_d[l, pr],
                    )
        for i in range(NB):
            nc.sync.dma_start(out=x32[i][:], in_=x_d[i * P:(i + 1) * P, :])
            if i % 2:
                nc.vector.tensor_copy(x16[0][i][:], x32[i][:])
            else:
                nc.scalar.activation(x16[0][i][:], x32[i][:], AF.Copy)

        with ExitStack() as bctx:
            # phase A pools
            spool = bctx.enter_context(tc.tile_pool(name="spool", bufs=2))
            tpool = bctx.enter_context(tc.tile_pool(name="tpool", bufs=2))
            vpool = bctx.enter_context(tc.tile_pool(name="vpool", bufs=3))
            tpsum = bctx.enter_context(tc.tile_pool(name="tpsum", bufs=2, space="PSUM"))
            # phase B pools
            ypsum = bctx.enter_context(tc.tile_pool(name="ypsum", bufs=3, space="PSUM"))
            zpsum = bctx.enter_context(tc.tile_pool(name="zpsum", bufs=2, space="PSUM"))
            lnpool = bctx.enter_context(tc.tile_pool(name="lnpool", bufs=2))
            smpool = bctx.enter_context(tc.tile_pool(name="smpool", bufs=8))

            def phase_a(m):
                lo, w = jlo(m) * P, wid(m)
                dm = (m - jlo(m)) * P  # local offset of the diagonal block
                sqx = tpool.tile([P, WMAX], F32)
                sqy = tpool.tile([P, WMAX], F32)
                nc.scalar.activation(sqx[:, :w], cxb_sb[:, lo:lo + w], AF.Square,
                                     bias=ccol_sb[:, m:m + 1])
                nc.scalar.activation(sqy[:, :w], cxb_sb[:, N + lo:N + lo + w],
                                     AF.Square, bias=ccol_sb[:, NB + m:NB + m + 1])
                srow = spool.tile([P, WMAX], F32)
                nc.vector.scalar_tensor_tensor(
                    srow[:, :w], sqx[:, :w], -1.0, sqy[:, :w],
                    ALU.mult, ALU.subtract)
                # mask self-distance (S[i,i]=0 is otherwise the row max)
                nc.gpsimd.affine_select(
                    out=srow[:, dm:dm + P], in_=srow[:, dm:dm + P],
                    pattern=[[1, P]], compare_op=ALU.not_equal,
                    fill=-1e9, base=0, channel_multiplier=-1,
                )
                v8 = vpool.tile([P, 8], F32)
                nc.vector.max(v8[:], srow[:, :w])
                nc.vector.tensor_scalar(
                    a_t[m][:], srow[:, :w], v8[:, 7:8], None, ALU.is_ge
                )

            def a2_diag(m):
                dm = (m - jlo(m)) * P
                td_ps = tpsum.tile([P, P], F16, tag="tp2")
                nc.tensor.transpose(td_ps[:], a_t[m][:, dm:dm + P], ident16[:])
                nc.vector.tensor_tensor(a_t[m][:, dm:dm + P],
                                        a_t[m][:, dm:dm + P], td_ps[:], ALU.max)
                nc.gpsimd.affine_select(
                    out=a_t[m][:, dm:dm + P], in_=a_t[m][:, dm:dm + P],
                    pattern=[[1, P]], compare_op=ALU.not_equal,
                    fill=0.0, base=0, channel_multiplier=-1,
                )

            def a2_pair(bi, bj):
                # up = A[bi][:, bj-block]; lo = A[bj][:, bi-block]
                du = (bj - jlo(bi)) * P
                dl = (bi - jlo(bj)) * P
                t1 = tpsum.tile([P, P], F16, tag="tp2")
                nc.tensor.transpose(t1[:], a_t[bj][:, dl:dl + P], ident16[:])
                nc.vector.tensor_tensor(a_t[bi][:, du:du + P],
                                        a_t[bi][:, du:du + P], t1[:], ALU.max)
                t2 = tpsum.tile([P, P], F16, tag="tp2")
                nc.tensor.transpose(t2[:], a_t[bi][:, du:du + P], ident16[:])
                nc.scalar.activation(a_t[bj][:, dl:dl + P], t2[:], AF.Copy)

            def layer_tile(l, bi):
                xr = x16[l % 2]
                xw = x16[(l + 1) % 2]
                ytl = yt[l % 2]
                jl, jh = jlo(bi), jhi(bi)
                y_ps = ypsum.tile([P, NDC, P], F32)
                for dt in range(NDC):
                    dcol = slice(dt * P, (dt + 1) * P)
                    for j in range(jl, jh + 1):
                        dj = (bi - jlo(j)) * P
                        nc.tensor.matmul(
                            y_ps[:, dt, :],
                            xr[j][:, dcol],
                            a_t[j][:, dj:dj + P],
                            start=(j == jl), stop=(j == jh),
                        )
                # yT[d, i] for this bi, all 4 d-chunks, fp32->fp16
                # (gpsimd cannot read PSUM; alternate ACT/DVE)
                if bi % 2:
                    nc.vector.tensor_copy(ytl[:, :, bi * P:(bi + 1) * P], y_ps[:])
                else:
                    nc.scalar.activation(ytl[:, :, bi * P:(bi + 1) * P], y_ps[:],
                                         AF.Copy)
                # z = yT^T @ W for this i-tile
                z_ps = zpsum.tile([P, D], F32)
                for dt in range(NDC):
                    nc.tensor.matmul(
                        z_ps[:],
                        ytl[:, dt, bi * P:(bi + 1) * P],
                        w_sb[:, (l * NDC + dt) * D:(l * NDC + dt + 1) * D],
                        start=(dt == 0), stop=(dt == NDC - 1),
                    )
                # LayerNorm via bn_stats; gamma==1, beta==0, b==0 fast path
                if triv:
                    u_ps = z_ps
                else:
                    boff = (l * 3 + 0) * D
                    zsb = lnpool.tile([P, D], F32)
                    nc.vector.tensor_tensor(zsb[:], z_ps[:],
                                            bgb_sb[:, boff:boff + D], ALU.add)
                    u_ps = zsb
                st6 = smpool.tile([P, 6], F32)
                nc.vector.bn_stats(st6[:], u_ps[:])
                mv = smpool.tile([P, 2], F32)
                nc.vector.bn_aggr(mv[:], st6[:])
                std = smpool.tile([P, 1], F32)
                nc.scalar.activation(std[:], mv[:, 1:2], AF.Sqrt, bias=eps_sb[:])
                rstd = smpool.tile([P, 1], F32)
                nc.vector.reciprocal(rstd[:], std[:])
                if triv:
                    negmr = smpool.tile([P, 1], F32)
                    nc.vector.tensor_scalar(negmr[:], mv[:, 0:1], rstd[:],
                                            -1.0, ALU.mult, ALU.mult)
                    if l == 0:
                        nc.scalar.activation(x32[bi][:], u_ps[:], AF.Relu,
                                             bias=negmr[:], scale=rstd[:])
                    else:
                        rt = lnpool.tile([P, D], F32)
                        nc.scalar.activation(rt[:], u_ps[:], AF.Relu,
                                             bias=negmr[:], scale=rstd[:])
                        nc.vector.tensor_tensor(x32[bi][:], x32[bi][:],
                                                rt[:], ALU.add)
                else:
                    goff = (l * 3 + 1) * D
                    toff = (l * 3 + 2) * D
                    xm = lnpool.tile([P, D], F32)
                    nc.vector.tensor_scalar(xm[:], u_ps[:], mv[:, 0:1],
                                            rstd[:], ALU.subtract, ALU.mult)
                    t2 = lnpool.tile([P, D], F32)
                    nc.vector.tensor_tensor(t2[:], xm[:],
                                            bgb_sb[:, goff:goff + D], ALU.mult)
                    u = lnpool.tile([P, D], F32)
                    nc.vector.tensor_tensor(u[:], t2[:],
                                            bgb_sb[:, toff:toff + D], ALU.add)
                    if l == 0:
                        nc.scalar.activation(x32[bi][:], u[:], AF.Relu)
                    else:
                        rt = lnpool.tile([P, D], F32)
                        nc.scalar.activation(rt[:], u[:], AF.Relu)
                        nc.vector.tensor_tensor(x32[bi][:], x32[bi][:],
                                                rt[:], ALU.add)
                if l < NLAYER - 1:
                    nc.gpsimd.tensor_copy(xw[bi][:], x32[bi][:])
                else:
                    nc.sync.dma_start(out=out_d[bi * P:(bi + 1) * P, :],
                                      in_=x32[bi][:])

            # interleaved phase A / A2 / layer 0
            for m in range(NB):
                phase_a(m)
                a2_diag(m)
                for j in range(jlo(m), m):
                    a2_pair(j, m)
                if m - 2 * bw >= 0:
                    layer_tile(0, m - 2 * bw)
            for bi in range(max(0, NB - 2 * bw), NB):
                layer_tile(0, bi)
            for l in range(1, NLAYER):
                for bi in range(NB):
                    layer_tile(l, bi)

    nc.compile()
    _CACHE[key] = nc
    return nc


def _band_width(coords_sorted):
    """Max |block_i - block_j| over kNN edges, per the device's own criterion."""
    bw = 1
    for b in range(coords_sorted.shape[0]):
        c = coords_sorted[b]
        dx = c[None, :, 0] - c[:, 0][:, None]
        dy = c[None, :, 1] - c[:, 1][:, None]
        S = -(dx * dx).astype(np.float32) - (dy * dy).astype(np.float32)
        np.fill_diagonal(S, -1e9)
        thr = np.partition(S, -8, axis=1)[:, -8]
        adj = S >= thr[:, None]
        ii, jj = np.nonzero(adj)
        bw = max(bw, int(np.abs(ii // P - jj // P).max()))
    return bw


def _host_inputs(node_features, coordinates, W, b, gamma, beta):
    """Per-core input dicts (host-side data marshaling only)."""
    w16 = np.ascontiguousarray(W.astype(np.float16).reshape(NLAYER, NDC, P, D))
    bgb = np.empty((NLAYER, 3, P, D), np.float32)
    for l in range(NLAYER):
        bgb[l, 0] = np.broadcast_to(b[l][None, :], (P, D))
        bgb[l, 1] = np.broadcast_to(gamma[l][None, :], (P, D))
        bgb[l, 2] = np.broadcast_to(beta[l][None, :], (P, D))
    in_maps = []
    perms = []
    cs_all = np.empty((B, N, 2), np.float32)
    for core in range(B):
        c = np.asarray(coordinates[core], dtype=np.float32)  # [N, 2]
        perm = np.argsort(c[:, 0], kind="stable")
        perms.append(perm)
        c = c[perm]
        cs_all[core] = c
        cxb = np.empty((2, P, N), np.float32)
        cxb[0] = np.broadcast_to(c[:, 0][None, :], (P, N))
        cxb[1] = np.broadcast_to(c[:, 1][None, :], (P, N))
        ccol = np.empty((2, P, NB), np.float32)
        ccol[0] = -c[:, 0].reshape(NB, P).T
        ccol[1] = -c[:, 1].reshape(NB, P).T
        in_maps.append({
            "xin": np.ascontiguousarray(
                np.asarray(node_features[core], dtype=np.float32)[perm]),
            "cxb": cxb,
            "ccol": ccol,
            "w16": w16,
            "bgb": bgb,
        })
    return in_maps, perms, cs_all


def kernel(node_features, coordinates, W, b, gamma, beta):
    triv = bool(np.all(b == 0) and np.all(gamma == 1) and np.all(beta == 0))
    in_maps, perms, cs_all = _host_inputs(node_features, coordinates, W, b,
                                          gamma, beta)
    bw = _band_width(cs_all)
    nc = _build_nc(bw=bw, triv=triv)
    res = bass_utils.run_bass_kernel_spmd(nc, in_maps, list(range(B)))
    out = np.empty((B, N, D), np.float32)
    for i in range(B):
        out[i][perms[i]] = res.results[i]["out"]
    return out


# revision 12
# speedup vs baseline: 1.0131x; 1.0131x over previous
"""Trainium2 Bass kernel for EnhancedFRAUnifiedEncoder (kNN-graph message passing).

Sharding: batch dim B=8 across 8 cores; each core runs the 3-layer GNN locally.
Host-side trick: nodes are sorted by x-coordinate per batch (the GNN is
permutation-equivariant), which makes the kNN adjacency block-tridiagonal
(bandwidth verified on host at runtime), so the [N,N] message-passing matmul,
the pairwise-distance build, and the symmetrization all shrink to the band.
Phase A (adjacency build), A2 (symmetrize), and layer-0 message passing are
interleaved per block-row so all engines stay busy from the start.
"""
import numpy as np
from contextlib import ExitStack

import concourse.tile as tile
from concourse import bacc, mybir
from concourse import bass_utils
from concourse.masks import make_identity

F32 = mybir.dt.float32
F16 = mybir.dt.float16
AF = mybir.ActivationFunctionType
ALU = mybir.AluOpType

B = 8
N = 2048
D = 512
P = 128
NB = N // P          # 16 row blocks
NDC = D // P         # 4 feature chunks of 128
NLAYER = 3
LN_EPS = 1e-5

_CACHE = {}


def _build_nc(bw=1, triv=True):
    """bw: adjacency half-bandwidth in 128-blocks. triv: b==0, gamma==1, beta==0."""
    key = ("nc", bw, triv)
    if key in _CACHE:
        return _CACHE[key]
    nc = bacc.Bacc("TRN2", target_bir_lowering=False, debug=False, num_devices=B)

    def jlo(m):
        return max(0, m - bw)

    def jhi(m):
        return min(NB - 1, m + bw)

    def wid(m):
        return (jhi(m) - jlo(m) + 1) * P

    WMAX = (2 * bw + 1) * P

    x_d = nc.dram_tensor("xin", [N, D], F32, kind="ExternalInput").ap()
    cxb_d = nc.dram_tensor("cxb", [2, P, N], F32, kind="ExternalInput").ap()
    ccol_d = nc.dram_tensor("ccol", [2, P, NB], F32, kind="ExternalInput").ap()
    w_d = nc.dram_tensor("w16", [NLAYER, NDC, P, D], F16, kind="ExternalInput").ap()
    bgb_d = nc.dram_tensor("bgb", [NLAYER, 3, P, D], F32, kind="ExternalInput").ap()
    out_d = nc.dram_tensor("out", [N, D], F32, kind="ExternalOutput").ap()

    with tile.TileContext(nc) as tc, ExitStack() as ctx:
        # ---- persistent pools -------------------------------------------------
        apool = ctx.enter_context(tc.tile_pool(name="apool", bufs=1))
        xpool = ctx.enter_context(tc.tile_pool(name="xpool", bufs=1))
        cpool = ctx.enter_context(tc.tile_pool(name="cpool", bufs=1))

        a_t = [apool.tile([P, wid(m)], F16, name=f"a{m}", tag=f"a{m}")
               for m in range(NB)]
        x32 = [xpool.tile([P, D], F32, name=f"x32_{i}", tag=f"x32_{i}")
               for i in range(NB)]
        # double-buffered fp16 x (layer l reads buf l%2) and yT
        x16 = [[xpool.tile([P, D], F16, name=f"x16_{s}_{i}", tag=f"x16_{s}_{i}")
                for i in range(NB)] for s in range(2)]
        yt = [xpool.tile([P, NDC, N], F16, name=f"yt{s}", tag=f"yt{s}")
              for s in range(2)]
        w_sb = cpool.tile([P, NLAYER * NDC * D], F16, name="w_sb", tag="w_sb")
        bgb_sb = (None if triv else
                  cpool.tile([P, NLAYER * 3 * D], F32, name="bgb_sb", tag="bgb_sb"))
        ident16 = cpool.tile([P, P], F16, name="ident16", tag="ident16")
        eps_sb = cpool.tile([P, 1], F32, name="eps_sb", tag="eps_sb")
        nc.gpsimd.memset(eps_sb[:], LN_EPS)
        make_identity(nc, ident16[:])

        # ---- input DMAs up front (overlap with phase A) ----------------------
        bpool = ctx.enter_context(tc.tile_pool(name="bpool", bufs=1))
        cxb_sb = bpool.tile([P, 2 * N], F32, name="cxb_sb", tag="cxb_sb")
        ccol_sb = bpool.tile([P, 2 * NB], F32, name="ccol_sb", tag="ccol_sb")
        nc.sync.dma_start(out=cxb_sb[:, 0:N], in_=cxb_d[0])
        nc.scalar.dma_start(out=cxb_sb[:, N:], in_=cxb_d[1])
        nc.sync.dma_start(out=ccol_sb[:, 0:NB], in_=ccol_d[0])
        nc.sync.dma_start(out=ccol_sb[:, NB:], in_=ccol_d[1])
        for l in range(NLAYER):
            for dt in range(NDC):
                nc.sync.dma_start(
                    out=w_sb[:, (l * NDC + dt) * D:(l * NDC + dt + 1) * D],
                    in_=w_d[l, dt],
                )
            if not triv:
                for pr in range(3):
                    nc.sync.dma_start(
                        out=bgb_sb[:, (l * 3 + pr) * D:(l * 3 + pr + 1) * D],
                        in_=bgb_d[l, pr],
                    )
        for i in range(NB):
            nc.sync.dma_start(out=x32[i][:], in_=x_d[i * P:(i + 1) * P, :])
            if i % 2:
                nc.vector.tensor_copy(x16[0][i][:], x32[i][:])
            else:
                nc.scalar.activation(x16[0][i][:], x32[i][:], AF.Copy)

        with ExitStack() as bctx:
            # phase A pools
            spool = bctx.enter_context(tc.tile_pool(name="spool", bufs=2))
            tpool = bctx.enter_context(tc.tile_pool(name="tpool", bufs=2))
            vpool = bctx.enter_context(tc.tile_pool(name="vpool", bufs=3))
            tpsum = bctx.enter_context(tc.tile_pool(name="tpsum", bufs=2, space="PSUM"))
            # phase B pools
            ypsum = bctx.enter_context(tc.tile_pool(name="ypsum", bufs=3, space="PSUM"))
            zpsum = bctx.enter_context(tc.tile_pool(name="zpsum", bufs=3, space="PSUM"))
            lnpool = bctx.enter_context(tc.tile_pool(name="lnpool", bufs=2))
            smpool = bctx.enter_context(tc.tile_pool(name="smpool", bufs=8))

            def phase_a(m):
                lo, w = jlo(m) * P, wid(m)
                dm = (m - jlo(m)) * P  # local offset of the diagonal block
                sqx = tpool.tile([P, WMAX], F32)
                sqy = tpool.tile([P, WMAX], F32)
                nc.scalar.activation(sqx[:, :w], cxb_sb[:, lo:lo + w], AF.Square,
                                     bias=ccol_sb[:, m:m + 1])
                nc.scalar.activation(sqy[:, :w], cxb_sb[:, N + lo:N + lo + w],
                                     AF.Square, bias=ccol_sb[:, NB + m:NB + m + 1])
                srow = spool.tile([P, WMAX], F32)
                nc.vector.scalar_tensor_tensor(
                    srow[:, :w], sqx[:, :w], -1.0, sqy[:, :w],
                    ALU.mult, ALU.subtract)
                # mask self-distance (S[i,i]=0 is otherwise the row max)
                nc.gpsimd.affine_select(
                    out=srow[:, dm:dm + P], in_=srow[:, dm:dm + P],
                    pattern=[[1, P]], compare_op=ALU.not_equal,
                    fill=-1e9, base=0, channel_multiplier=-1,
                )
                v8 = vpool.tile([P, 8], F32)
                nc.vector.max(v8[:], srow[:, :w])
                nc.vector.tensor_scalar(
                    a_t[m][:], srow[:, :w], v8[:, 7:8], None, ALU.is_ge
                )

            def a2_diag(m):
                dm = (m - jlo(m)) * P
                td_ps = tpsum.tile([P, P], F16, tag="tp2")
                nc.tensor.transpose(td_ps[:], a_t[m][:, dm:dm + P], ident16[:])
                nc.vector.tensor_tensor(a_t[m][:, dm:dm + P],
                                        a_t[m][:, dm:dm + P], td_ps[:], ALU.max)
                nc.gpsimd.affine_select(
                    out=a_t[m][:, dm:dm + P], in_=a_t[m][:, dm:dm + P],
                    pattern=[[1, P]], compare_op=ALU.not_equal,
                    fill=0.0, base=0, channel_multiplier=-1,
                )

            def a2_pair(bi, bj):
                # up = A[bi][:, bj-block]; lo = A[bj][:, bi-block]
                du = (bj - jlo(bi)) * P
                dl = (bi - jlo(bj)) * P
                t1 = tpsum.tile([P, P], F16, tag="tp2")
                nc.tensor.transpose(t1[:], a_t[bj][:, dl:dl + P], ident16[:])
                nc.vector.tensor_tensor(a_t[bi][:, du:du + P],
                                        a_t[bi][:, du:du + P], t1[:], ALU.max)
                t2 = tpsum.tile([P, P], F16, tag="tp2")
                nc.tensor.transpose(t2[:], a_t[bi][:, du:du + P], ident16[:])
                nc.scalar.activation(a_t[bj][:, dl:dl + P], t2[:], AF.Copy)

            def layer_tile(l, bi):
                xr = x16[l % 2]
                xw = x16[(l + 1) % 2]
                ytl = yt[l % 2]
                jl, jh = jlo(bi), jhi(bi)
                y_ps = ypsum.tile([P, NDC, P], F32)
                for dt in range(NDC):
                    dcol = slice(dt * P, (dt + 1) * P)
                    for j in range(jl, jh + 1):
                        dj = (bi - jlo(j)) * P
                        nc.tensor.matmul(
                            y_ps[:, dt, :],
                            xr[j][:, dcol],
                            a_t[j][:, dj:dj + P],
                            start=(j == jl), stop=(j == jh),
                        )
                # yT[d, i] for this bi, all 4 d-chunks, fp32->fp16
                nc.vector.tensor_copy(ytl[:, :, bi * P:(bi + 1) * P], y_ps[:])
                # z = yT^T @ W for this i-tile
                z_ps = zpsum.tile([P, D], F32)
                for dt in range(NDC):
                    nc.tensor.matmul(
                        z_ps[:],
                        ytl[:, dt, bi * P:(bi + 1) * P],
                        w_sb[:, (l * NDC + dt) * D:(l * NDC + dt + 1) * D],
                        start=(dt == 0), stop=(dt == NDC - 1),
                    )
                # LayerNorm via bn_stats; gamma==1, beta==0, b==0 fast path
                if triv:
                    u_ps = z_ps
                else:
                    boff = (l * 3 + 0) * D
                    zsb = lnpool.tile([P, D], F32)
                    nc.vector.tensor_tensor(zsb[:], z_ps[:],
                                            bgb_sb[:, boff:boff + D], ALU.add)
                    u_ps = zsb
                st6 = smpool.tile([P, 6], F32)
                nc.vector.bn_stats(st6[:], u_ps[:])
                mv = smpool.tile([P, 2], F32)
                nc.vector.bn_aggr(mv[:], st6[:])
                std = smpool.tile([P, 1], F32)
                nc.scalar.activation(std[:], mv[:, 1:2], AF.Sqrt, bias=eps_sb[:])
                rstd = smpool.tile([P, 1], F32)
                nc.vector.reciprocal(rstd[:], std[:])
                if triv:
                    negmr = smpool.tile([P, 1], F32)
                    nc.vector.tensor_scalar(negmr[:], mv[:, 0:1], rstd[:],
                                            -1.0, ALU.mult, ALU.mult)
                    if l == 0:
                        nc.scalar.activation(x32[bi][:], u_ps[:], AF.Relu,
                                             bias=negmr[:], scale=rstd[:])
                    else:
                        rt = lnpool.tile([P, D], F32)
                        nc.scalar.activation(rt[:], u_ps[:], AF.Relu,
                                             bias=negmr[:], scale=rstd[:])
                        nc.vector.tensor_tensor(x32[bi][:], x32[bi][:],
                                                rt[:], ALU.add)
                else:
                    goff = (l * 3 + 1) * D
                    toff = (l * 3 + 2) * D
                    xm = lnpool.tile([P, D], F32)
                    nc.vector.tensor_scalar(xm[:], u_ps[:], mv[:, 0:1],
                                            rstd[:], ALU.subtract, ALU.mult)
                    t2 = lnpool.tile([P, D], F32)
                    nc.vector.tensor_tensor(t2[:], xm[:],
                                            bgb_sb[:, goff:goff + D], ALU.mult)
                    u = lnpool.tile([P, D], F32)
                    nc.vector.tensor_tensor(u[:], t2[:],
                                            bgb_sb[:, toff:toff + D], ALU.add)
                    if l == 0:
                        nc.scalar.activation(x32[bi][:], u[:], AF.Relu)
                    else:
                        rt = lnpool.tile([P, D], F32)
                        nc.scalar.activation(rt[:], u[:], AF.Relu)
                        nc.vector.tensor_tensor(x32[bi][:], x32[bi][:],
                                                rt[:], ALU.add)
                if l < NLAYER - 1:
                    if bi % 2:
                        nc.vector.tensor_copy(xw[bi][:], x32[bi][:])
                    else:
                        nc.scalar.activation(xw[bi][:], x32[bi][:], AF.Copy)
                else:
                    nc.sync.dma_start(out=out_d[bi * P:(bi + 1) * P, :],
                                      in_=x32[bi][:])

            # interleaved phase A / A2 / layer 0
            for m in range(NB):
                phase_a(m)
                a2_diag(m)
                for j in range(jlo(m), m):
                    a2_pair(j, m)
                if m - 2 * bw >= 0:
                    layer_tile(0, m - 2 * bw)
            for bi in range(max(0, NB - 2 * bw), NB):
                layer_tile(0, bi)
            for l in range(1, NLAYER):
                for bi in range(NB):
                    layer_tile(l, bi)

    nc.compile()
    _CACHE[key] = nc
    return nc


def _band_width(coords_sorted):
    """Max |block_i - block_j| over kNN edges, per the device's own criterion."""
    bw = 1
    for b in range(coords_sorted.shape[0]):
        c = coords_sorted[b]
        dx = c[None, :, 0] - c[:, 0][:, None]
        dy = c[None, :, 1] - c[:, 1][:, None]
        S = -(dx * dx).astype(np.float32) - (dy * dy).astype(np.float32)
        np.fill_diagonal(S, -1e9)
        thr = np.partition(S, -8, axis=1)[:, -8]
        adj = S >= thr[:, None]
        ii, jj = np.nonzero(adj)
        bw = max(bw, int(np.abs(ii // P - jj // P).max()))
    return bw


def _host_inputs(node_features, coordinates, W, b, gamma, beta):
    """Per-core input dicts (host-side data marshaling only)."""
    w16 = np.ascontiguousarray(W.astype(np.float16).reshape(NLAYER, NDC, P, D))
    bgb = np.empty((NLAYER, 3, P, D), np.float32)
    for l in range(NLAYER):
        bgb[l, 0] = np.broadcast_to(b[l][None, :], (P, D))
        bgb[l, 1] = np.broadcast_to(gamma[l][None, :], (P, D))
        bgb[l, 2] = np.broadcast_to(beta[l][None, :], (P, D))
    in_maps = []
    perms = []
    cs_all = np.empty((B, N, 2), np.float32)
    for core in range(B):
        c = np.asarray(coordinates[core], dtype=np.float32)  # [N, 2]
        perm = np.argsort(c[:, 0], kind="stable")
        perms.append(perm)
        c = c[perm]
        cs_all[core] = c
        cxb = np.empty((2, P, N), np.float32)
        cxb[0] = np.broadcast_to(c[:, 0][None, :], (P, N))
        cxb[1] = np.broadcast_to(c[:, 1][None, :], (P, N))
        ccol = np.empty((2, P, NB), np.float32)
        ccol[0] = -c[:, 0].reshape(NB, P).T
        ccol[1] = -c[:, 1].reshape(NB, P).T
        in_maps.append({
            "xin": np.ascontiguousarray(
                np.asarray(node_features[core], dtype=np.float32)[perm]),
            "cxb": cxb,
            "ccol": ccol,
            "w16": w16,
            "bgb": bgb,
        })
    return in_maps, perms, cs_all


def kernel(node_features, coordinates, W, b, gamma, beta):
    triv = bool(np.all(b == 0) and np.all(gamma == 1) and np.all(beta == 0))
    in_maps, perms, cs_all = _host_inputs(node_features, coordinates, W, b,
                                          gamma, beta)
    bw = _band_width(cs_all)
    nc = _build_nc(bw=bw, triv=triv)
    res = bass_utils.run_bass_kernel_spmd(nc, in_maps, list(range(B)))
    out = np.empty((B, N, D), np.float32)
    for i in range(B):
        out[i][perms[i]] = res.results[i]["out"]
    return out
